# revision 2
# baseline (speedup 1.0000x reference)
"""Trainium2 Bass kernel for nn_Actor (topk_masking).

Reference semantics:
    s    = einsum('ol,bld->bod', W, state)[:, 0, :]        # (B, D) weighted sum over L
    a0   = softmax(s, axis=-1)
    loop T-1 times: zero the argmax entry, renormalize; stack all T states
    out  = (B, T, D)

Key identity used here: after t steps the state equals a0 with its top-t
entries zeroed, divided by the sum of the remaining entries.  Working with
unnormalized e = exp(w * sum_l x_l):
    out[t] = (e with top-t entries zeroed) * C_t,   C_t = 1 / D_t
    D_t    = sum of remaining entries of e = (suffix sum of top-24 values
             v_{t+1..24}) + R,  R = sum of e with top-24 zeroed.
D_t is computed from small positive sums (never S - T) so it stays accurate
even when the top values carry almost all of the mass.

Output rows are written to DRAM in fp16 (the 2e-2 harness tolerance has
~40x margin over fp16 rounding; top-k ORDER is decided entirely in f32 so
no rank swaps are introduced) and upcast to f32 on the host.  This halves
store traffic.  Loads are issued from the Sync engine's HWDGE queue and
stores from the Scalar engine's queue so the two streams round-robin at
the SDMA packet level instead of serializing in one ring.

Sharding: pure data parallel over the batch dim across 8 NeuronCores.
"""

import numpy as np

from concourse import bacc, bass, mybir
from concourse import tile as tile_mod
from concourse.tile import add_dep_helper
from concourse.bass_utils import run_bass_kernel_spmd

F32 = mybir.dt.float32
F16 = mybir.dt.float16
AF = mybir.ActivationFunctionType
ALU = mybir.AluOpType

# Problem constants (hardcoded per harness contract)
B_FULL = 4096
L = 50
D = 1000
T = 20
N_CORES = 8
B_CORE = B_FULL // N_CORES  # 512
P = 128                     # partitions per tile
N_TILES = B_CORE // P       # 4

# L-chunking for the streaming load: megatiles of MEGA_L l-slices each.
MEGA_L = 5
N_MEGA = L // MEGA_L  # 10
N_GP = 4  # megatiles summed on GPSIMD (its own chain); rest on DVE
MID_MR_ANCHOR = 8  # next tile sum chain may start after this mr step
ROWS_PER_STORE = 4  # output rows batched per store DMA (1.02 MB fp16)


def build_graph(b_core=B_CORE, w_vals=None, uniform_w=1.0):
    """Build the per-core Bass graph.

    If w_vals is None, the fast path is used: s = uniform_w * sum_l x_l and
    the scalar uniform_w is folded into the exp.  Otherwise w_vals is a list
    of L floats baked in as immediates for a general weighted sum.
    """
    n_tiles = b_core // P
    nc = bacc.Bacc("TRN2")
    state_ext = nc.declare_dram_parameter("state", [b_core, L, D], F32, isOutput=False)
    out_ext = nc.declare_dram_parameter("out", [b_core, T, D], F16, isOutput=True)

    with tile_mod.TileContext(nc) as tc:
        with (
            tc.tile_pool(name="mega", bufs=5) as mega_pool,
            tc.tile_pool(name="work", bufs=2) as work_pool,
            tc.tile_pool(name="epool", bufs=7) as e_pool,
            tc.tile_pool(name="rows", bufs=3) as row_pool,
            tc.tile_pool(name="small", bufs=2) as small_pool,
        ):
            prev_tile_last_dve = None
            for bt in range(n_tiles):
                b0 = bt * P
                first_dve_of_tile = None

                # ---- stream state in; sum over L with contiguous adds ----
                # (strided tensor_reduce runs at 2 cycles/elem; contiguous
                # tensor_tensor adds run at 1 — so sum via chains of 5-wide
                # contiguous adds, then fold 5 -> 1.)
                # GPSIMD owns an independent chain over N_GP megas plus the
                # final merge (loaded first so its slower adds start early);
                # DVE owns the rest.
                megas = [None] * N_MEGA
                gp_idx = list(range(N_MEGA - N_GP, N_MEGA))
                dve_idx = list(range(N_MEGA - N_GP))
                # Load order: first two DVE megas (so the DVE chain starts
                # immediately), then the GPSIMD megas (its chain is slower),
                # then the rest.
                if bt == 0:
                    # first tile: get the DVE chain going immediately
                    load_order = dve_idx[:2] + gp_idx + dve_idx[2:]
                else:
                    load_order = gp_idx + dve_idx
                for mi in load_order:
                    M = mega_pool.tile([P, MEGA_L, D], F32, tag="mega")
                    nc.sync.dma_start(
                        M[:],
                        state_ext[
                            b0 : b0 + P, mi * MEGA_L : (mi + 1) * MEGA_L, :
                        ],
                    )
                    megas[mi] = M

                if w_vals is not None:
                    # general path: scale each mega by its per-l weights
                    # in place before summing (w broadcast per l-slice).
                    for mi, M in enumerate(megas):
                        for li in range(MEGA_L):
                            wl = float(w_vals[mi * MEGA_L + li])
                            sc = nc.vector.tensor_scalar(
                                M[:, li, :], M[:, li, :], wl, None, ALU.mult
                            )
                            if first_dve_of_tile is None:
                                first_dve_of_tile = sc
                                if prev_tile_last_dve is not None:
                                    add_dep_helper(
                                        sc.ins, prev_tile_last_dve.ins,
                                        sync=False,
                                        reason="tile interleave on DVE",
                                    )

                # GPSIMD chain over gp_idx megas (in-place accumulation into
                # its first mega: ~10% slower per op on GPSIMD but frees an
                # SBUF buffer pool slot)
                b_cur = megas[gp_idx[0]]
                for mi in gp_idx[1:]:
                    nc.gpsimd.tensor_tensor(
                        b_cur[:], b_cur[:], megas[mi][:], ALU.add
                    )

                # DVE chain over dve_idx megas
                w_cur = None
                for mi in dve_idx[1:]:
                    w_nxt = work_pool.tile([P, MEGA_L, D], F32, tag="w5")
                    lhs = megas[dve_idx[0]][:] if w_cur is None else w_cur[:]
                    addi = nc.vector.tensor_tensor(
                        w_nxt[:], lhs, megas[mi][:], ALU.add
                    )
                    # Interleave tiles on DVE: this tile's sum chain starts
                    # only mid-way through the previous tile's row chain, so
                    # DMA loads and stores spread over the whole kernel but
                    # sum-adds can still fill DVE idle slots.
                    if first_dve_of_tile is None:
                        first_dve_of_tile = addi
                        if prev_tile_last_dve is not None:
                            add_dep_helper(
                                addi.ins, prev_tile_last_dve.ins,
                                sync=False,
                                reason="tile interleave on DVE",
                            )
                    w_cur = w_nxt

                # merge the two chains on GPSIMD (in place into its acc),
                # freeing DVE for top-k / row work
                nc.gpsimd.tensor_tensor(b_cur[:], b_cur[:], w_cur[:], ALU.add)

                # fold [P, 5, D] -> [P, D] with in-place accumulation (DVE)
                partial = work_pool.tile([P, D], F32, tag="s")
                nc.vector.tensor_tensor(
                    partial[:], b_cur[:, 0, :], b_cur[:, 1, :], ALU.add
                )
                for j in range(2, MEGA_L):
                    nc.vector.tensor_tensor(
                        partial[:], partial[:], b_cur[:, j, :], ALU.add
                    )

                # ---- e = exp(scale * s) ----
                e0 = e_pool.tile([P, D], F32, tag="e")
                scale = uniform_w if w_vals is None else 1.0
                nc.scalar.activation(e0[:], partial[:], AF.Exp, bias=0.0, scale=scale)

                # ---- extract top-24 values, R = sum of the rest ----
                # One consolidated stats tile:
                #   [0:31]  v_pad: 7 sentinels (-1) + 24 top values (desc)
                #   [32:56] suf:   suffix sums of the top values
                #   [56:76] D      [76:96] C      [96:97] R
                st = small_pool.tile([P, 104], F32, tag="stats")
                v_pad = st[:, 0:31]
                suf = st[:, 32:56]
                Dt = st[:, 56:76]
                Ct = st[:, 76:96]
                R = st[:, 96:97]
                nc.vector.memset(v_pad[:, 0:7], -1.0)
                va = v_pad[:, 7:15]
                vb = v_pad[:, 15:23]
                vc = v_pad[:, 23:31]
                u = work_pool.tile([P, D], F32, tag="u")
                nc.vector.max(va, e0[:])
                nc.vector.match_replace(u[:], va, e0[:], 0.0)
                nc.vector.max(vb, u[:])
                # Seed tiles for four independent row chains:
                #   e5  = e0 with top-5 zeroed (window: 3 sentinels + v1..v5)
                #   e10 = u (top-8 zeroed) with v9, v10 also zeroed
                #   e15 = e10 with v11..v15 also zeroed
                e5 = e_pool.tile([P, D], F32, tag="e")
                nc.vector.match_replace(e5[:], v_pad[:, 4:12], e0[:], 0.0)
                e10 = e_pool.tile([P, D], F32, tag="e")
                nc.vector.match_replace(e10[:], v_pad[:, 9:17], u[:], 0.0)
                e15 = e_pool.tile([P, D], F32, tag="e")
                nc.vector.match_replace(e15[:], v_pad[:, 14:22], e10[:], 0.0)
                nc.vector.match_replace(u[:], vb, u[:], 0.0)
                nc.vector.max(vc, u[:])
                nc.vector.match_replace(u[:], vc, u[:], 0.0)
                nc.vector.tensor_reduce(
                    R, u[:], axis=mybir.AxisListType.X, op=ALU.add
                )

                # ---- D_t = suffix_sum(v_{t+1..24}) + R ;  C = 1/D ----
                # suf[:, j] = v_24 + ... + v_{24-j}  (cumsum of reversed v)
                v_rev = v_pad[:, 30:6:-1]  # v_24, v_23, ..., v_1
                nc.vector.tensor_tensor_scan(
                    suf, v_rev, v_rev, 0.0, ALU.add, ALU.bypass
                )
                # D[:, t] = suf[:, 23 - t] + R  for t = 0..19
                nc.vector.tensor_scalar(
                    Dt, suf[:, 23:3:-1], R, None, ALU.add
                )
                nc.vector.reciprocal(Ct, Dt)

                # ---- emit rows; zero one more top value between rows ----
                # Four independent chains of 5 rows each (seeds e0, e5,
                # e10, e15) cut the serial mr-chain latency to ~1/4.  Rows
                # are staged (fp16) in groups and stored with one wide DMA
                # per group, issued from the Scalar engine's HWDGE queue so
                # stores don't queue behind loads.  A group may span two
                # chains; rows are emitted in t order so the store (issued
                # at the group's last row) is always last.
                T_Q = T // 4
                rowgs = {}
                for t_base, e_seed in (
                    (0, e0), (T_Q, e5), (2 * T_Q, e10), (3 * T_Q, e15)
                ):
                    e_cur = e_seed
                    for t in range(t_base, t_base + T_Q):
                        g = t // ROWS_PER_STORE
                        j = t % ROWS_PER_STORE
                        if g not in rowgs:
                            rowgs[g] = row_pool.tile(
                                [P, ROWS_PER_STORE, D], F16, tag="rowg",
                                name=f"rowg_{bt}_{g}",
                            )
                        nc.scalar.activation(
                            rowgs[g][:, j, :], e_cur[:], AF.Copy, bias=0.0,
                            scale=Ct[:, t : t + 1],
                        )
                        if j == ROWS_PER_STORE - 1:
                            t0 = t - j
                            nc.scalar.dma_start(
                                out_ext[b0 : b0 + P, t0 : t + 1, :],
                                rowgs[g][:],
                            )
                        if t < t_base + T_Q - 1:
                            e_nxt = e_pool.tile([P, D], F32, tag="e")
                            # window: 7 zeroed/sentinel values + v_{t+1}
                            mr = nc.vector.match_replace(
                                e_nxt[:], v_pad[:, t : t + 8], e_cur[:], 0.0
                            )
                            if t == MID_MR_ANCHOR:
                                prev_tile_last_dve = mr
                            e_cur = e_nxt

    nc.finalize()
    return nc


_GRAPH_CACHE = {}


def _get_graph(w):
    w = np.asarray(w, dtype=np.float32).reshape(-1)
    assert w.shape[0] == L
    if np.all(w == w[0]):
        key = ("uniform", float(w[0]))
        if key not in _GRAPH_CACHE:
            _GRAPH_CACHE[key] = build_graph(w_vals=None, uniform_w=float(w[0]))
    else:
        key = ("general", tuple(float(x) for x in w))
        if key not in _GRAPH_CACHE:
            _GRAPH_CACHE[key] = build_graph(w_vals=[float(x) for x in w])
    return _GRAPH_CACHE[key]


def kernel(state, weight_matrix):
    state = np.ascontiguousarray(np.asarray(state, dtype=np.float32))
    w = np.asarray(weight_matrix, dtype=np.float32)
    assert state.shape == (B_FULL, L, D), state.shape

    nc = _get_graph(w)
    in_maps = [
        {"state": state[i * B_CORE : (i + 1) * B_CORE]} for i in range(N_CORES)
    ]
    res = run_bass_kernel_spmd(nc, in_maps, core_ids=list(range(N_CORES)))
    out = np.concatenate([res.results[i]["out"] for i in range(N_CORES)], axis=0)
    return out.astype(np.float32)


# revision 5
# speedup vs baseline: 1.2224x; 1.2224x over previous
"""Trainium2 Bass kernel for nn_Actor (topk_masking).

Reference semantics:
    s    = einsum('ol,bld->bod', W, state)[:, 0, :]        # (B, D) weighted sum over L
    a0   = softmax(s, axis=-1)
    loop T-1 times: zero the argmax entry, renormalize; stack all T states
    out  = (B, T, D)

Key identity used here: after t steps the state equals a0 with its top-t
entries zeroed, divided by the sum of the remaining entries.  Working with
unnormalized e = exp(w * sum_l x_l):
    out[t] = (e with top-t entries zeroed) * C_t,   C_t = 1 / D_t
    D_t    = sum of remaining entries of e = (suffix sum of top-24 values
             v_{t+1..24}) + R,  R = sum of e with top-24 zeroed.
D_t is computed from small positive sums (never S - T) so it stays accurate
even when the top values carry almost all of the mass.

Output rows are written to DRAM in fp16 (the 2e-2 harness tolerance has
~40x margin over fp16 rounding; top-k ORDER is decided entirely in f32 so
no rank swaps are introduced) and upcast to f32 on the host.  This halves
store traffic.  Loads are issued from the Sync engine's HWDGE queue and
stores from the Scalar engine's queue so the two streams round-robin at
the SDMA packet level instead of serializing in one ring.

Sharding: pure data parallel over the batch dim across 8 NeuronCores.
"""

import numpy as np

from concourse import bacc, bass, mybir
from concourse import tile as tile_mod
from concourse.tile import add_dep_helper
from concourse.bass_utils import run_bass_kernel_spmd

F32 = mybir.dt.float32
F16 = mybir.dt.float16
AF = mybir.ActivationFunctionType
ALU = mybir.AluOpType

# Problem constants (hardcoded per harness contract)
B_FULL = 4096
L = 50
D = 1000
T = 20
N_CORES = 8
B_CORE = B_FULL // N_CORES  # 512
P = 128                     # partitions per tile
N_TILES = B_CORE // P       # 4

# L-chunking for the streaming load: megatiles of MEGA_L l-slices each.
MEGA_L = 5
N_MEGA = L // MEGA_L  # 10
N_GP = 3  # megatiles summed on GPSIMD (its own chain); rest on DVE
MID_MR_ANCHOR = 4  # next tile sum chain may start after this mr step
ROWS_PER_STORE = 4  # output rows batched per store DMA (1.02 MB fp16)


def build_graph(b_core=B_CORE, w_vals=None, uniform_w=1.0):
    """Build the per-core Bass graph.

    If w_vals is None, the fast path is used: s = uniform_w * sum_l x_l and
    the scalar uniform_w is folded into the exp.  Otherwise w_vals is a list
    of L floats baked in as immediates for a general weighted sum.
    """
    n_tiles = b_core // P
    nc = bacc.Bacc("TRN2")
    state_ext = nc.declare_dram_parameter("state", [b_core, L, D], F32, isOutput=False)
    out_ext = nc.declare_dram_parameter("out", [b_core, T, D], F16, isOutput=True)

    with tile_mod.TileContext(nc) as tc:
        with (
            tc.tile_pool(name="mega", bufs=5) as mega_pool,
            tc.tile_pool(name="work", bufs=2) as work_pool,
            tc.tile_pool(name="epool", bufs=7) as e_pool,
            tc.tile_pool(name="rows", bufs=3) as row_pool,
            tc.tile_pool(name="small", bufs=2) as small_pool,
        ):
            prev_tile_last_dve = None
            for bt in range(n_tiles):
                b0 = bt * P
                first_dve_of_tile = None

                # ---- stream state in; sum over L with contiguous adds ----
                # (strided tensor_reduce runs at 2 cycles/elem; contiguous
                # tensor_tensor adds run at 1 — so sum via chains of 5-wide
                # contiguous adds, then fold 5 -> 1.)
                # GPSIMD owns an independent chain over N_GP megas plus the
                # final merge (loaded first so its slower adds start early);
                # DVE owns the rest.
                megas = [None] * N_MEGA
                gp_idx = list(range(N_MEGA - N_GP, N_MEGA))
                dve_idx = list(range(N_MEGA - N_GP))
                # Load order: first two DVE megas (so the DVE chain starts
                # immediately), then the GPSIMD megas (its chain is slower),
                # then the rest.
                if bt == 0:
                    # first tile: interleave so BOTH chains stream from the
                    # start (a block of gp loads would starve the DVE chain
                    # for ~size_of_block of DMA time)
                    load_order = dve_idx[:2]
                    rest = dve_idx[2:]
                    for k, g in enumerate(gp_idx):
                        load_order.append(g)
                        if k < len(rest):
                            load_order.append(rest[k])
                    load_order += rest[len(gp_idx):]
                else:
                    load_order = gp_idx + dve_idx
                for mi in load_order:
                    M = mega_pool.tile([P, MEGA_L, D], F32, tag="mega")
                    nc.sync.dma_start(
                        M[:],
                        state_ext[
                            b0 : b0 + P, mi * MEGA_L : (mi + 1) * MEGA_L, :
                        ],
                    )
                    megas[mi] = M

                if w_vals is not None:
                    # general path: scale each mega by its per-l weights
                    # in place before summing (w broadcast per l-slice).
                    for mi, M in enumerate(megas):
                        for li in range(MEGA_L):
                            wl = float(w_vals[mi * MEGA_L + li])
                            sc = nc.vector.tensor_scalar(
                                M[:, li, :], M[:, li, :], wl, None, ALU.mult
                            )
                            if first_dve_of_tile is None:
                                first_dve_of_tile = sc
                                if prev_tile_last_dve is not None:
                                    add_dep_helper(
                                        sc.ins, prev_tile_last_dve.ins,
                                        sync=False,
                                        reason="tile interleave on DVE",
                                    )

                # GPSIMD chain over gp_idx megas (in-place accumulation into
                # its first mega: ~10% slower per op on GPSIMD but frees an
                # SBUF buffer pool slot)
                b_cur = megas[gp_idx[0]]
                for mi in gp_idx[1:]:
                    nc.gpsimd.tensor_tensor(
                        b_cur[:], b_cur[:], megas[mi][:], ALU.add
                    )

                # DVE chain over dve_idx megas
                w_cur = None
                for mi in dve_idx[1:]:
                    w_nxt = work_pool.tile([P, MEGA_L, D], F32, tag="w5")
                    lhs = megas[dve_idx[0]][:] if w_cur is None else w_cur[:]
                    addi = nc.vector.tensor_tensor(
                        w_nxt[:], lhs, megas[mi][:], ALU.add
                    )
                    # Interleave tiles on DVE: this tile's sum chain starts
                    # only mid-way through the previous tile's row chain, so
                    # DMA loads and stores spread over the whole kernel but
                    # sum-adds can still fill DVE idle slots.
                    if first_dve_of_tile is None:
                        first_dve_of_tile = addi
                        if prev_tile_last_dve is not None:
                            add_dep_helper(
                                addi.ins, prev_tile_last_dve.ins,
                                sync=False,
                                reason="tile interleave on DVE",
                            )
                    w_cur = w_nxt

                # merge the two chains (DVE — gpsimd elementwise adds steal
                # DVE SBUF-port bandwidth, so keep its share small)
                w_mrg = work_pool.tile([P, MEGA_L, D], F32, tag="w5")
                nc.vector.tensor_tensor(w_mrg[:], w_cur[:], b_cur[:], ALU.add)

                # fold [P, 5, D] -> [P, D] with in-place accumulation (DVE)
                partial = work_pool.tile([P, D], F32, tag="s")
                nc.vector.tensor_tensor(
                    partial[:], w_mrg[:, 0, :], w_mrg[:, 1, :], ALU.add
                )
                for j in range(2, MEGA_L):
                    nc.vector.tensor_tensor(
                        partial[:], partial[:], w_mrg[:, j, :], ALU.add
                    )

                # ---- e = exp(scale * s) ----
                e0 = e_pool.tile([P, D], F32, tag="e")
                scale = uniform_w if w_vals is None else 1.0
                nc.scalar.activation(e0[:], partial[:], AF.Exp, bias=0.0, scale=scale)

                # ---- extract top-24 values, R = sum of the rest ----
                # One consolidated stats tile:
                #   [0:31]  v_pad: 7 sentinels (-1) + 24 top values (desc)
                #   [32:56] suf:   suffix sums of the top values
                #   [56:76] D      [76:96] C      [96:97] R
                st = small_pool.tile([P, 104], F32, tag="stats")
                v_pad = st[:, 0:31]
                suf = st[:, 32:56]
                Dt = st[:, 56:76]
                Ct = st[:, 76:96]
                R = st[:, 96:97]
                nc.vector.memset(v_pad[:, 0:7], -1.0)
                va = v_pad[:, 7:15]
                vb = v_pad[:, 15:23]
                vc = v_pad[:, 23:31]
                u = work_pool.tile([P, D], F32, tag="u")
                nc.vector.max(va, e0[:])
                nc.vector.match_replace(u[:], va, e0[:], 0.0)
                nc.vector.max(vb, u[:])
                # Seed tiles for four independent row chains:
                #   e5  = e0 with top-5 zeroed (window: 3 sentinels + v1..v5)
                #   e10 = u (top-8 zeroed) with v9, v10 also zeroed
                #   e15 = e10 with v11..v15 also zeroed
                e5 = e_pool.tile([P, D], F32, tag="e")
                nc.vector.match_replace(e5[:], v_pad[:, 4:12], e0[:], 0.0)
                e10 = e_pool.tile([P, D], F32, tag="e")
                nc.vector.match_replace(e10[:], v_pad[:, 9:17], u[:], 0.0)
                e15 = e_pool.tile([P, D], F32, tag="e")
                nc.vector.match_replace(e15[:], v_pad[:, 14:22], e10[:], 0.0)
                nc.vector.match_replace(u[:], vb, u[:], 0.0)
                nc.vector.max(vc, u[:])
                nc.vector.match_replace(u[:], vc, u[:], 0.0)
                nc.vector.tensor_reduce(
                    R, u[:], axis=mybir.AxisListType.X, op=ALU.add
                )

                # ---- D_t = suffix_sum(v_{t+1..24}) + R ;  C = 1/D ----
                # suf[:, j] = v_24 + ... + v_{24-j}  (cumsum of reversed v)
                v_rev = v_pad[:, 30:6:-1]  # v_24, v_23, ..., v_1
                nc.vector.tensor_tensor_scan(
                    suf, v_rev, v_rev, 0.0, ALU.add, ALU.bypass
                )
                # D[:, t] = suf[:, 23 - t] + R  for t = 0..19
                nc.vector.tensor_scalar(
                    Dt, suf[:, 23:3:-1], R, None, ALU.add
                )
                nc.vector.reciprocal(Ct, Dt)

                # ---- emit rows; zero one more top value between rows ----
                # Four independent chains of 5 rows each (seeds e0, e5,
                # e10, e15) cut the serial mr-chain latency to ~1/4.  Rows
                # are staged (fp16) in groups and stored with one wide DMA
                # per group, issued from the Scalar engine's HWDGE queue so
                # stores don't queue behind loads.  A group may span two
                # chains; rows are emitted in t order so the store (issued
                # at the group's last row) is always last.
                T_Q = T // 4
                rowgs = {}
                for t_base, e_seed in (
                    (0, e0), (T_Q, e5), (2 * T_Q, e10), (3 * T_Q, e15)
                ):
                    e_cur = e_seed
                    for t in range(t_base, t_base + T_Q):
                        g = t // ROWS_PER_STORE
                        j = t % ROWS_PER_STORE
                        if g not in rowgs:
                            rowgs[g] = row_pool.tile(
                                [P, ROWS_PER_STORE, D], F16, tag="rowg",
                                name=f"rowg_{bt}_{g}",
                            )
                        nc.scalar.activation(
                            rowgs[g][:, j, :], e_cur[:], AF.Copy, bias=0.0,
                            scale=Ct[:, t : t + 1],
                        )
                        if j == ROWS_PER_STORE - 1:
                            t0 = t - j
                            nc.scalar.dma_start(
                                out_ext[b0 : b0 + P, t0 : t + 1, :],
                                rowgs[g][:],
                            )
                        if t < t_base + T_Q - 1:
                            e_nxt = e_pool.tile([P, D], F32, tag="e")
                            # window: 7 zeroed/sentinel values + v_{t+1}
                            mr = nc.vector.match_replace(
                                e_nxt[:], v_pad[:, t : t + 8], e_cur[:], 0.0
                            )
                            if t == MID_MR_ANCHOR:
                                prev_tile_last_dve = mr
                            e_cur = e_nxt

    nc.finalize()
    return nc


_GRAPH_CACHE = {}


def _get_graph(w):
    w = np.asarray(w, dtype=np.float32).reshape(-1)
    assert w.shape[0] == L
    if np.all(w == w[0]):
        key = ("uniform", float(w[0]))
        if key not in _GRAPH_CACHE:
            _GRAPH_CACHE[key] = build_graph(w_vals=None, uniform_w=float(w[0]))
    else:
        key = ("general", tuple(float(x) for x in w))
        if key not in _GRAPH_CACHE:
            _GRAPH_CACHE[key] = build_graph(w_vals=[float(x) for x in w])
    return _GRAPH_CACHE[key]


def kernel(state, weight_matrix):
    state = np.ascontiguousarray(np.asarray(state, dtype=np.float32))
    w = np.asarray(weight_matrix, dtype=np.float32)
    assert state.shape == (B_FULL, L, D), state.shape

    nc = _get_graph(w)
    in_maps = [
        {"state": state[i * B_CORE : (i + 1) * B_CORE]} for i in range(N_CORES)
    ]
    res = run_bass_kernel_spmd(nc, in_maps, core_ids=list(range(N_CORES)))
    out = np.concatenate([res.results[i]["out"] for i in range(N_CORES)], axis=0)
    return out.astype(np.float32)
